# revision 11
# baseline (speedup 1.0000x reference)
"""Trainium2 Bass kernel for CustomMoE (top-2 routing, E=8 experts, expert parallel).

Contract: kernel(**inputs) takes FULL unsharded inputs (x, wg, w1, b1, w2, b2, k)
and returns (output [T, D] fp32, l_aux scalar fp32) matching reference().

Strategy (8 NeuronCores, expert parallelism — one expert per core):
  - routing replicated on every core (cheap), computed in token-major layout
  - each core gathers the tokens routed to its expert into capacity slots
    (C_PAD=1280 slots; mean load is 1024, so 8.5-sigma headroom; tokens are
    dropped exactly like the reference iff any load exceeded C_PAD, which is
    unreachable for randn-style inputs), runs the FFN on its expert weights
    in fp16 (fp22 accumulate), scatters gate-scaled results back per token,
  - AllReduce over the 8 cores combines the per-expert partial outputs.
"""

import numpy as np
import ml_dtypes

import concourse.bass as bass
import concourse.bacc as bacc
import concourse.mybir as mybir
import concourse.tile as tile
from concourse.bass_utils import run_bass_kernel_spmd
from concourse.masks import make_identity

f32 = mybir.dt.float32
f16 = mybir.dt.float16
bf16 = mybir.dt.bfloat16
i32 = mybir.dt.int32
AO = mybir.AluOpType
ACT = mybir.ActivationFunctionType

T, D, H, E, K = 4096, 2048, 8192, 8, 2
NB = T // 128            # 32 token blocks (token t = b*128 + p)
C_PAD = 1280             # capacity slots materialized per expert (>= max load)
NC_CHUNKS = C_PAD // 128  # 10
DK = D // 128            # 16 contraction chunks over D
HQ = H // 128            # 64 chunks over H
N_CORES = 8

_BUILD_CACHE = {}


def build_bass():
    if "nc" in _BUILD_CACHE:
        return _BUILD_CACHE["nc"]
    nc = bacc.Bacc(None)

    # ---- inputs (per core; routing inputs identical on all cores) ----
    xt_hi = nc.dram_tensor("xt_hi", [16, 8, 128, 512], bf16, kind="ExternalInput")
    xt_lo = nc.dram_tensor("xt_lo", [16, 8, 128, 512], bf16, kind="ExternalInput")
    wg_hi = nc.dram_tensor("wg_hi", [D, E], bf16, kind="ExternalInput")
    wg_lo = nc.dram_tensor("wg_lo", [D, E], bf16, kind="ExternalInput")
    x16 = nc.dram_tensor("x16", [T, D], f16, kind="ExternalInput")
    w1c = nc.dram_tensor("w1c", [DK, HQ, 128, 128], f16, kind="ExternalInput")
    w2c = nc.dram_tensor("w2c", [HQ, 4, 128, 512], f16, kind="ExternalInput")
    b1v = nc.dram_tensor("b1v", [H], f32, kind="ExternalInput")
    b2v = nc.dram_tensor("b2v", [D], f32, kind="ExternalInput")
    eidt = nc.dram_tensor("eidt", [128, 1], f32, kind="ExternalInput")

    # ---- outputs ----
    out = nc.dram_tensor("out", [T, D], f32, kind="ExternalOutput")
    laux = nc.dram_tensor("laux", [1], f32, kind="ExternalOutput")

    # ---- internal DRAM ----
    tokmapd = nc.dram_tensor("tokmapd", [C_PAD + 128, 16], i32)
    y_dram = nc.dram_tensor("y_dram", [C_PAD + 1, D], f16)
    partial_d = nc.dram_tensor("partial_d", [T, D], f32)
    ar_out = nc.dram_tensor("ar_out", [T, D], f32, addr_space="Shared")

    with tile.TileContext(nc) as tc:
        with (
            tc.tile_pool(name="const", bufs=1) as cpool,
            tc.tile_pool(name="rout", bufs=1) as rp,
            tc.tile_pool(name="persist", bufs=1) as pp,
        ):
            # ================= constants =================
            ident16 = cpool.tile([128, 128], f16)
            make_identity(nc, ident16[:])
            ident32 = cpool.tile([128, 128], f32)
            make_identity(nc, ident32[:])
            ut128 = cpool.tile([128, 128], f32)
            nc.vector.memset(ut128[:], 1.0)
            nc.gpsimd.affine_select(  # keep where freeidx >= partition
                out=ut128[:], in_=ut128[:], compare_op=AO.is_ge, fill=0.0,
                base=0, pattern=[[1, 128]], channel_multiplier=-1,
            )
            ones_col = cpool.tile([128, 1], f32)
            nc.vector.memset(ones_col[:], 1.0)
            ones_1x128 = cpool.tile([1, 128], f32)
            nc.vector.memset(ones_1x128[:], 1.0)
            ones_8 = cpool.tile([128, 8], f32)
            nc.vector.memset(ones_8[:], 1.0)
            ones_row256 = cpool.tile([1, 256], f32)
            nc.vector.memset(ones_row256[:], 1.0)
            eid = cpool.tile([128, 1], f32)
            nc.sync.dma_start(out=eid[:], in_=eidt[:])
            # expert index per free-slot: [128, 32, 8] values 0..7
            eidx_i = cpool.tile([128, 256], i32)
            nc.gpsimd.iota(eidx_i[:], pattern=[[0, 32], [1, 8]], base=0,
                           channel_multiplier=0)
            eidx = cpool.tile([128, 256], f32)
            nc.vector.tensor_copy(eidx[:], eidx_i[:])
            # token id per (p, b): t = b*128 + p, replicated x16 for scatter rows
            tiota_i = cpool.tile([128, 32, 16], i32)
            nc.gpsimd.iota(tiota_i[:], pattern=[[128, 32], [0, 16]], base=0,
                           channel_multiplier=1)
            # wg chunks [128, 16, 8]
            wgh = cpool.tile([128, DK, E], bf16)
            nc.sync.dma_start(out=wgh[:], in_=wg_hi.rearrange("(k p) e -> p k e", p=128))
            wgl = cpool.tile([128, DK, E], bf16)
            nc.sync.dma_start(out=wgl[:], in_=wg_lo.rearrange("(k p) e -> p k e", p=128))

            # ================= PHASE R: routing =================
            lg = rp.tile([128, 256], f32)  # logits token-major [128, b=32, e=8]
            with (
                tc.tile_pool(name="lgp", bufs=2, space="PSUM") as lgp,
                tc.tile_pool(name="ltp", bufs=2, space="PSUM") as ltp,
                tc.tile_pool(name="lsb", bufs=2) as lsb,
                tc.tile_pool(name="xts", bufs=4) as xts,
            ):
                for g in range(8):
                    ps_lg = lgp.tile([8, 512], f32)
                    for dk in range(DK):
                        xh = xts.tile([128, 512], bf16, tag="xh")
                        nc.sync.dma_start(out=xh[:], in_=xt_hi[dk, g])
                        xl = xts.tile([128, 512], bf16, tag="xl")
                        nc.sync.dma_start(out=xl[:], in_=xt_lo[dk, g])
                        wh_s = wgh[:, dk, :]
                        wl_s = wgl[:, dk, :]
                        nc.tensor.matmul(ps_lg[:], wh_s, xh[:], start=(dk == 0), stop=False)
                        nc.tensor.matmul(ps_lg[:], wl_s, xh[:], start=False, stop=False)
                        nc.tensor.matmul(ps_lg[:], wh_s, xl[:], start=False,
                                         stop=(dk == DK - 1))
                    lg_sb = lsb.tile([8, 512], f32)
                    nc.scalar.copy(lg_sb[:], ps_lg[:])
                    for q in range(4):
                        ps_t = ltp.tile([128, 8], f32)
                        nc.tensor.transpose(ps_t[:], lg_sb[:, q * 128:(q + 1) * 128],
                                            ident32[0:8, 0:8])
                        b = 4 * g + q
                        nc.vector.tensor_copy(lg[:, b * 8:(b + 1) * 8], ps_t[:])

            lg3 = lg[:].rearrange("p (b e) -> p b e", e=8)
            # --- top-2 on raw logits ---
            m1 = rp.tile([128, 32], f32)
            nc.vector.tensor_reduce(m1[:], lg3, axis=mybir.AxisListType.X, op=AO.max)
            oh0 = rp.tile([128, 256], f32)
            oh03 = oh0[:].rearrange("p (b e) -> p b e", e=8)
            for b in range(NB):
                nc.vector.tensor_scalar(oh03[:, b, :], lg3[:, b, :], m1[:, b:b + 1],
                                        None, AO.is_equal)
            # tie-break: keep only first match per token
            ohs = rp.tile([128, 256], f32)
            ohs3 = ohs[:].rearrange("p (b e) -> p b e", e=8)
            for b in range(NB):
                nc.vector.tensor_tensor_scan(ohs3[:, b, :], ones_8[:], oh03[:, b, :],
                                             0.0, AO.mult, AO.add)
            tmp = rp.tile([128, 256], f32)
            nc.vector.tensor_scalar(tmp[:], ohs[:], 1.0, None, AO.is_equal)
            nc.vector.tensor_mul(oh0[:], oh0[:], tmp[:])
            # masked logits -> m2, oh1
            lgm = rp.tile([128, 256], f32)
            nc.vector.scalar_tensor_tensor(lgm[:], oh0[:], -1e30, lg[:], AO.mult, AO.add)
            lgm3 = lgm[:].rearrange("p (b e) -> p b e", e=8)
            m2 = rp.tile([128, 32], f32)
            nc.vector.tensor_reduce(m2[:], lgm3, axis=mybir.AxisListType.X, op=AO.max)
            oh1 = rp.tile([128, 256], f32)
            oh13 = oh1[:].rearrange("p (b e) -> p b e", e=8)
            for b in range(NB):
                nc.vector.tensor_scalar(oh13[:, b, :], lgm3[:, b, :], m2[:, b:b + 1],
                                        None, AO.is_equal)
            for b in range(NB):
                nc.vector.tensor_tensor_scan(ohs3[:, b, :], ones_8[:], oh13[:, b, :],
                                             0.0, AO.mult, AO.add)
            nc.vector.tensor_scalar(tmp[:], ohs[:], 1.0, None, AO.is_equal)
            nc.vector.tensor_mul(oh1[:], oh1[:], tmp[:])

            # --- softmax pieces (max-subtracted) ---
            ex = rp.tile([128, 256], f32)
            ex3 = ex[:].rearrange("p (b e) -> p b e", e=8)
            for b in range(NB):
                nc.vector.tensor_scalar(ex3[:, b, :], lg3[:, b, :], m1[:, b:b + 1],
                                        None, AO.subtract)
            nc.scalar.activation(ex[:], ex[:], ACT.Exp)
            sumex = rp.tile([128, 32], f32)
            nc.vector.tensor_reduce(sumex[:], ex3, axis=mybir.AxisListType.X, op=AO.add)
            rec = rp.tile([128, 32], f32)
            nc.vector.reciprocal(rec[:], sumex[:])
            # gates: g0 = 1*rec ; g1 = exp(m2-m1)*rec
            d21 = rp.tile([128, 32], f32)
            nc.vector.tensor_sub(d21[:], m2[:], m1[:])
            nc.scalar.activation(d21[:], d21[:], ACT.Exp)
            g1t = rp.tile([128, 32], f32)
            nc.vector.tensor_mul(g1t[:], d21[:], rec[:])
            # normalized scores (for l_aux's me)
            S = rp.tile([128, 256], f32)
            S3 = S[:].rearrange("p (b e) -> p b e", e=8)
            for b in range(NB):
                nc.vector.tensor_scalar(S3[:, b, :], ex3[:, b, :], rec[:, b:b + 1],
                                        None, AO.mult)
            # expert indices of top1/top2
            idx0 = rp.tile([128, 32], f32)
            nc.vector.tensor_mul(tmp[:], oh0[:], eidx[:])
            nc.vector.tensor_reduce(idx0[:], tmp[:].rearrange("p (b e) -> p b e", e=8),
                                    axis=mybir.AxisListType.X, op=AO.add)
            idx1 = rp.tile([128, 32], f32)
            nc.vector.tensor_mul(tmp[:], oh1[:], eidx[:])
            nc.vector.tensor_reduce(idx1[:], tmp[:].rearrange("p (b e) -> p b e", e=8),
                                    axis=mybir.AxisListType.X, op=AO.add)

            # --- cumsum over token order (t = b*128 + p) ---
            with (
                tc.tile_pool(name="csp", bufs=2, space="PSUM") as cspp,
                tc.tile_pool(name="cump", bufs=2, space="PSUM") as cumpp,
            ):
                # per-(b, e) column sums  [1, 256]
                ps_cs0 = cspp.tile([1, 256], f32)
                nc.tensor.matmul(ps_cs0[:], ones_col[:], oh0[:], start=True, stop=True)
                css0 = rp.tile([1, 256], f32)
                nc.vector.tensor_copy(css0[:], ps_cs0[:])
                ps_cs1 = cspp.tile([1, 256], f32)
                nc.tensor.matmul(ps_cs1[:], ones_col[:], oh1[:], start=True, stop=True)
                css1 = rp.tile([1, 256], f32)
                nc.vector.tensor_copy(css1[:], ps_cs1[:])
                # inclusive prefix over b per e (scan over strided [1, 32] views)
                sc0 = rp.tile([1, 256], f32)
                sc03 = sc0[:].rearrange("o (b e) -> o e b", e=8)
                css03 = css0[:].rearrange("o (b e) -> o e b", e=8)
                for e in range(E):
                    nc.vector.tensor_tensor_scan(sc03[:, e, :], ones_row256[:, 0:32],
                                                 css03[:, e, :], 0.0, AO.mult, AO.add)
                sc1 = rp.tile([1, 256], f32)
                sc13 = sc1[:].rearrange("o (b e) -> o e b", e=8)
                css13 = css1[:].rearrange("o (b e) -> o e b", e=8)
                for e in range(E):
                    nc.vector.tensor_tensor_scan(sc13[:, e, :], ones_row256[:, 0:32],
                                                 css13[:, e, :], 0.0, AO.mult, AO.add)
                # exclusive prefixes
                pre0 = rp.tile([1, 256], f32)
                nc.vector.tensor_sub(pre0[:], sc0[:], css0[:])
                pre1 = rp.tile([1, 256], f32)
                nc.vector.tensor_sub(pre1[:], sc1[:], css1[:])
                # slot-1 positions come after ALL slot-0 assignments:
                # add total0[e] (= sc0 at b=31) broadcast over b
                tot0 = sc0[0:1, 248:256]  # [1, 8]
                pre1b = pre1[0:1, :].rearrange("o (b e) -> o b e", e=8)
                nc.vector.tensor_tensor(pre1b, pre1b,
                                        tot0.rearrange("o (u e) -> o u e", u=1)
                                        .to_broadcast([1, 32, 8]),
                                        AO.add)
                # cum (inclusive within-expert position + 1) = UT@oh + bcast(prefix)
                cum0 = rp.tile([128, 256], f32)
                ps_c0 = cumpp.tile([128, 256], f32)
                nc.tensor.matmul(ps_c0[:], ut128[:], oh0[:], start=True, stop=False)
                nc.tensor.matmul(ps_c0[:], ones_1x128[:], pre0[:], start=False, stop=True)
                nc.vector.tensor_copy(cum0[:], ps_c0[:])
                cum1 = rp.tile([128, 256], f32)
                ps_c1 = cumpp.tile([128, 256], f32)
                nc.tensor.matmul(ps_c1[:], ut128[:], oh1[:], start=True, stop=False)
                nc.tensor.matmul(ps_c1[:], ones_1x128[:], pre1[:], start=False, stop=True)
                nc.vector.tensor_copy(cum1[:], ps_c1[:])

            # per-token slot position within its expert (0-based)
            c0 = rp.tile([128, 32], f32)
            nc.vector.tensor_mul(tmp[:], cum0[:], oh0[:])
            nc.vector.tensor_reduce(c0[:], tmp[:].rearrange("p (b e) -> p b e", e=8),
                                    axis=mybir.AxisListType.X, op=AO.add)
            nc.vector.tensor_scalar(c0[:], c0[:], 1.0, None, AO.subtract)
            c1 = rp.tile([128, 32], f32)
            nc.vector.tensor_mul(tmp[:], cum1[:], oh1[:])
            nc.vector.tensor_reduce(c1[:], tmp[:].rearrange("p (b e) -> p b e", e=8),
                                    axis=mybir.AxisListType.X, op=AO.add)
            nc.vector.tensor_scalar(c1[:], c1[:], 1.0, None, AO.subtract)

            # --- per-core (expert e = eid) selection ---
            sel0 = rp.tile([128, 32], f32)
            nc.vector.tensor_scalar(sel0[:], idx0[:], eid[:, 0:1], None, AO.is_equal)
            sel1 = rp.tile([128, 32], f32)
            nc.vector.tensor_scalar(sel1[:], idx1[:], eid[:, 0:1], None, AO.is_equal)
            kept0 = rp.tile([128, 32], f32)
            nc.vector.tensor_scalar(kept0[:], c0[:], float(C_PAD), None, AO.is_lt)
            nc.vector.tensor_mul(kept0[:], kept0[:], sel0[:])
            kept1 = rp.tile([128, 32], f32)
            nc.vector.tensor_scalar(kept1[:], c1[:], float(C_PAD), None, AO.is_lt)
            nc.vector.tensor_mul(kept1[:], kept1[:], sel1[:])
            # coeff = kept0*g0 + kept1*g1   (g0 = rec)
            coeff = pp.tile([128, 32], f32)
            nc.vector.tensor_mul(coeff[:], kept0[:], rec[:])
            t2 = rp.tile([128, 32], f32)
            nc.vector.tensor_mul(t2[:], kept1[:], g1t[:])
            nc.vector.tensor_add(coeff[:], coeff[:], t2[:])
            # slotof = kept0*c0 + kept1*c1 + (1-kept0-kept1)*C_PAD
            slotof = rp.tile([128, 32], f32)
            nc.vector.tensor_mul(slotof[:], kept0[:], c0[:])
            nc.vector.tensor_mul(t2[:], kept1[:], c1[:])
            nc.vector.tensor_add(slotof[:], slotof[:], t2[:])
            kk = rp.tile([128, 32], f32)
            nc.vector.tensor_add(kk[:], kept0[:], kept1[:])
            nc.vector.scalar_tensor_tensor(slotof[:], kk[:], -float(C_PAD), slotof[:],
                                           AO.mult, AO.add)
            nc.vector.tensor_scalar(slotof[:], slotof[:], float(C_PAD), None, AO.add)
            slot_i = pp.tile([128, 32], i32)
            nc.vector.tensor_copy(slot_i[:], slotof[:])

            # --- l_aux ---
            with tc.tile_pool(name="lap", bufs=1, space="PSUM") as lap:
                ps_me = lap.tile([1, 256], f32)
                nc.tensor.matmul(ps_me[:], ones_col[:], S[:], start=True, stop=True)
                me_b = rp.tile([1, 256], f32)
                nc.vector.tensor_copy(me_b[:], ps_me[:])
            me8 = rp.tile([1, 8], f32)
            nc.vector.tensor_reduce(me8[:], me_b[:].rearrange("o (b e) -> o e b", e=8),
                                    axis=mybir.AxisListType.X, op=AO.add)
            ce8 = rp.tile([1, 8], f32)
            nc.vector.tensor_reduce(ce8[:], css0[:].rearrange("o (b e) -> o e b", e=8),
                                    axis=mybir.AxisListType.X, op=AO.add)
            nc.vector.tensor_mul(me8[:], me8[:], ce8[:])
            la = rp.tile([1, 1], f32)
            nc.vector.tensor_reduce(la[:], me8[:], axis=mybir.AxisListType.X, op=AO.add)
            nc.vector.tensor_scalar(la[:], la[:], float(E) / (T * T), None, AO.mult)
            nc.sync.dma_start(out=laux[:], in_=la[:])

            # --- tokmap scatter: tokmapd[slot] = token id ---
            zrow = rp.tile([128, 11, 16], i32)
            nc.vector.memset(zrow[:], 0)
            nc.sync.dma_start(
                out=tokmapd.rearrange("(ci p) w -> p ci w", p=128), in_=zrow[:])
            for g in range(NB):
                nc.gpsimd.indirect_dma_start(
                    out=tokmapd[:], out_offset=bass.IndirectOffsetOnAxis(
                        ap=slot_i[:, g:g + 1], axis=0),
                    in_=tiota_i[:, g, :], in_offset=None,
                )

            # ================= PHASE E: encode (gather + transpose) ===========
            dispT = pp.tile([128, DK * C_PAD], f16)  # [d-chunk][128d, 1280c]
            tokmap_sb = pp.tile([128, NC_CHUNKS], i32)
            nc.sync.dma_start(
                out=tokmap_sb[:],
                in_=tokmapd.rearrange("(ci p) w -> p ci w", p=128)[:, 0:NC_CHUNKS, 0])
            with (
                tc.tile_pool(name="disp", bufs=3) as dp,
                tc.tile_pool(name="trp", bufs=4, space="PSUM") as trp,
            ):
                for ci in range(NC_CHUNKS):
                    dchunk = dp.tile([128, D], f16, tag="dchunk")
                    nc.gpsimd.indirect_dma_start(
                        out=dchunk[:], out_offset=None,
                        in_=x16[:], in_offset=bass.IndirectOffsetOnAxis(
                            ap=tokmap_sb[:, ci:ci + 1], axis=0),
                    )
                    for dk in range(DK):
                        ps_tr = trp.tile([128, 128], f16)
                        nc.tensor.transpose(ps_tr[:], dchunk[:, dk * 128:(dk + 1) * 128],
                                            ident16[:])
                        nc.vector.tensor_copy(
                            dispT[:, dk * C_PAD + ci * 128: dk * C_PAD + (ci + 1) * 128],
                            ps_tr[:])

            # ================= PHASE F: FFN =================
            b1c = cpool.tile([128, HQ], f32)
            nc.sync.dma_start(out=b1c[:], in_=b1v.rearrange("(hq p) -> p hq", p=128))
            b2row = cpool.tile([1, D], f32)
            nc.sync.dma_start(out=b2row[:], in_=b2v.rearrange("(u d) -> u d", u=1))
            zero16 = cpool.tile([1, D], f16)
            nc.vector.memset(zero16[:], 0.0)
            nc.sync.dma_start(out=y_dram[C_PAD:C_PAD + 1, :], in_=zero16[:])

            c_sups = [(0, 512), (512, 512), (1024, 256)]
            with (
                tc.tile_pool(name="hT", bufs=1) as hpool,
                tc.tile_pool(name="w1s", bufs=24) as w1p,
                tc.tile_pool(name="w2s", bufs=6) as w2p,
                tc.tile_pool(name="psh", bufs=2, space="PSUM") as pshp,
                tc.tile_pool(name="psy", bufs=1, space="PSUM") as psyp,
                tc.tile_pool(name="ysb", bufs=4) as yp,
            ):
                hT = hpool.tile([128, HQ * 512], f16)
                for cs, cw in c_sups:
                    # FFN1: h_T[h', c] = relu(sum_d w1[d, h'] * dispT[d, c] + b1)
                    for hq in range(HQ):
                        ps_h = pshp.tile([128, 512], f32, tag="ps_h")
                        for dk in range(DK):
                            w1t = w1p.tile([128, 128], f16, tag="w1t")
                            nc.sync.dma_start(out=w1t[:], in_=w1c[dk, hq])
                            nc.tensor.matmul(
                                ps_h[:, 0:cw], w1t[:],
                                dispT[:, dk * C_PAD + cs: dk * C_PAD + cs + cw],
                                start=(dk == 0), stop=(dk == DK - 1))
                        nc.scalar.activation(hT[:, hq * 512: hq * 512 + cw],
                                             ps_h[:, 0:cw], ACT.Relu,
                                             bias=b1c[:, hq:hq + 1])
                    # FFN2: y[c, d] = sum_h' h_T[h', c] * w2[h', d] + b2
                    ncc = cw // 128
                    for dg in range(4):
                        ps_ys = [psyp.tile([128, 512], f32, tag=f"ps_y{cc}",
                                           name=f"ps_y{cc}")
                                 for cc in range(ncc)]
                        for hk in range(HQ):
                            w2t = w2p.tile([128, 512], f16, tag="w2t")
                            nc.sync.dma_start(out=w2t[:], in_=w2c[hk, dg])
                            for cc in range(ncc):
                                nc.tensor.matmul(
                                    ps_ys[cc][:],
                                    hT[:, hk * 512 + cc * 128: hk * 512 + (cc + 1) * 128],
                                    w2t[:],
                                    start=(hk == 0), stop=False)
                        for cc in range(ncc):
                            nc.tensor.matmul(ps_ys[cc][:], ones_1x128[:],
                                             b2row[:, dg * 512:(dg + 1) * 512],
                                             start=False, stop=True)
                            y_sb = yp.tile([128, 512], f16, tag="y_sb")
                            nc.vector.tensor_copy(y_sb[:], ps_ys[cc][:])
                            nc.sync.dma_start(
                                out=y_dram[cs + cc * 128: cs + (cc + 1) * 128,
                                           dg * 512:(dg + 1) * 512],
                                in_=y_sb[:])

            # ================= PHASE D: decode =================
            with (
                tc.tile_pool(name="yt", bufs=3) as ytp,
                tc.tile_pool(name="op", bufs=3) as opp,
            ):
                for g in range(NB):
                    yt = ytp.tile([128, D], f16, tag="yt")
                    nc.gpsimd.indirect_dma_start(
                        out=yt[:], out_offset=None,
                        in_=y_dram[:], in_offset=bass.IndirectOffsetOnAxis(
                            ap=slot_i[:, g:g + 1], axis=0),
                    )
                    op_t = opp.tile([128, D], f32, tag="op_t")
                    nc.vector.tensor_scalar(op_t[:], yt[:], coeff[:, g:g + 1],
                                            None, AO.mult)
                    nc.sync.dma_start(out=partial_d[g * 128:(g + 1) * 128, :],
                                      in_=op_t[:])

            # ================= PHASE C: AllReduce =================
            nc.gpsimd.collective_compute(
                "AllReduce", AO.add,
                replica_groups=[list(range(N_CORES))],
                ins=[partial_d[:]], outs=[ar_out[:]],
            )
            nc.sync.dma_start(out=out[:], in_=ar_out[:])

    nc.compile()
    _BUILD_CACHE["nc"] = nc
    return nc


def _prep_inputs(x, wg, w1, b1, w2, b2):
    x = np.asarray(x, np.float32)
    wg = np.asarray(wg, np.float32)
    w1 = np.asarray(w1, np.float32)
    b1 = np.asarray(b1, np.float32)
    w2 = np.asarray(w2, np.float32)
    b2 = np.asarray(b2, np.float32)

    xT = np.ascontiguousarray(x.T)  # [D, T]
    xt_hi = xT.astype(ml_dtypes.bfloat16)
    xt_lo = (xT - xt_hi.astype(np.float32)).astype(ml_dtypes.bfloat16)
    # chunk layout [dk, g, 128, 512]
    xt_hi = np.ascontiguousarray(
        xt_hi.reshape(16, 128, 8, 512).transpose(0, 2, 1, 3))
    xt_lo = np.ascontiguousarray(
        xt_lo.reshape(16, 128, 8, 512).transpose(0, 2, 1, 3))
    wg_hi = wg.astype(ml_dtypes.bfloat16)
    wg_lo = (wg - wg_hi.astype(np.float32)).astype(ml_dtypes.bfloat16)
    x16 = x.astype(np.float16)

    shared = {
        "xt_hi": xt_hi, "xt_lo": xt_lo,
        "wg_hi": wg_hi, "wg_lo": wg_lo, "x16": x16,
    }
    in_maps = []
    for e in range(N_CORES):
        w1e = w1[e].astype(np.float16)  # [D, H]
        w1ce = np.ascontiguousarray(
            w1e.reshape(16, 128, 64, 128).transpose(0, 2, 1, 3))
        w2e = w2[e].astype(np.float16)  # [H, D]
        w2ce = np.ascontiguousarray(
            w2e.reshape(64, 128, 4, 512).transpose(0, 2, 1, 3))
        m = dict(shared)
        m["w1c"] = w1ce
        m["w2c"] = w2ce
        m["b1v"] = np.ascontiguousarray(b1[e, 0])
        m["b2v"] = np.ascontiguousarray(b2[e, 0])
        m["eidt"] = np.full((128, 1), float(e), np.float32)
        in_maps.append(m)
    return in_maps


def kernel(x, wg, w1, b1, w2, b2, k=2, _want_results=False, _trace=False, **_ignored):
    assert int(k) == K
    nc = build_bass()
    in_maps = _prep_inputs(x, wg, w1, b1, w2, b2)
    res = run_bass_kernel_spmd(nc, in_maps, core_ids=list(range(N_CORES)),
                               trace=_trace)
    out = np.asarray(res.results[0]["out"])
    laux = np.float32(np.asarray(res.results[0]["laux"])[0])
    if _want_results:
        return (out, laux), res
    return out, laux


# revision 17
# speedup vs baseline: 1.5206x; 1.5206x over previous
"""Trainium2 Bass kernel for CustomMoE (top-2 routing, E=8 experts, expert parallel).

Contract: kernel(**inputs) takes FULL unsharded inputs (x, wg, w1, b1, w2, b2, k)
and returns (output [T, D] fp32, l_aux scalar fp32) matching reference().

Strategy (8 NeuronCores, expert parallelism — one expert per core):
  - routing replicated on every core (cheap), computed in token-major layout
  - each core gathers the tokens routed to its expert into capacity slots
    (C_PAD=1280 slots; mean load is 1024, so 8.5-sigma headroom; tokens are
    dropped exactly like the reference iff any load exceeded C_PAD, which is
    unreachable for randn-style inputs), runs the FFN on its expert weights
    in fp16 (fp22 accumulate), scatters gate-scaled results back per token,
  - AllReduce over the 8 cores combines the per-expert partial outputs.
"""

import numpy as np
import ml_dtypes

import concourse.bass as bass
import concourse.bacc as bacc
import concourse.mybir as mybir
import concourse.tile as tile
from concourse.bass_utils import run_bass_kernel_spmd
from concourse.masks import make_identity

f32 = mybir.dt.float32
f16 = mybir.dt.float16
bf16 = mybir.dt.bfloat16
i32 = mybir.dt.int32
AO = mybir.AluOpType
ACT = mybir.ActivationFunctionType

T, D, H, E, K = 4096, 2048, 8192, 8, 2
NB = T // 128            # 32 token blocks (token t = b*128 + p)
C_PAD = 1280             # capacity slots materialized per expert (>= max load)
NC_CHUNKS = C_PAD // 128  # 10
DK = D // 128            # 16 contraction chunks over D
HQ = H // 128            # 64 chunks over H
N_CORES = 8

_BUILD_CACHE = {}


def build_bass():
    if "nc" in _BUILD_CACHE:
        return _BUILD_CACHE["nc"]
    nc = bacc.Bacc(None)

    # ---- inputs (per core; routing inputs identical on all cores) ----
    xt_hi = nc.dram_tensor("xt_hi", [16, 8, 128, 512], bf16, kind="ExternalInput")
    xt_lo = nc.dram_tensor("xt_lo", [16, 8, 128, 512], bf16, kind="ExternalInput")
    wg_hi = nc.dram_tensor("wg_hi", [D, E], bf16, kind="ExternalInput")
    wg_lo = nc.dram_tensor("wg_lo", [D, E], bf16, kind="ExternalInput")
    x16 = nc.dram_tensor("x16", [T, D], f16, kind="ExternalInput")
    w1c = nc.dram_tensor("w1c", [DK, HQ // 4, 128, 512], f16, kind="ExternalInput")
    w2c = nc.dram_tensor("w2c", [HQ, 4, 128, 512], f16, kind="ExternalInput")
    b1v = nc.dram_tensor("b1v", [H], f32, kind="ExternalInput")
    b2v = nc.dram_tensor("b2v", [D], f32, kind="ExternalInput")
    eidt = nc.dram_tensor("eidt", [128, 1], f32, kind="ExternalInput")

    # ---- outputs ----
    out = nc.dram_tensor("out", [T, D], f32, kind="ExternalOutput")
    laux = nc.dram_tensor("laux", [1], f32, kind="ExternalOutput")

    # ---- internal DRAM ----
    tokmapd = nc.dram_tensor("tokmapd", [C_PAD + 128, 16], i32)
    y_dram = nc.dram_tensor("y_dram", [C_PAD + 1, D], f16)
    partial_d = nc.dram_tensor("partial_d", [T, D], f32)
    ar_out = nc.dram_tensor("ar_out", [T, D], f32, addr_space="Shared")

    with tile.TileContext(nc) as tc:
        with (
            tc.tile_pool(name="const", bufs=1) as cpool,
            tc.tile_pool(name="rout", bufs=1) as rp,
            tc.tile_pool(name="persist", bufs=1) as pp,
        ):
            # ================= constants =================
            ident16 = cpool.tile([128, 128], f16)
            make_identity(nc, ident16[:])
            ident32 = cpool.tile([128, 128], f32)
            make_identity(nc, ident32[:])
            ut128 = cpool.tile([128, 128], f32)
            nc.vector.memset(ut128[:], 1.0)
            nc.gpsimd.affine_select(  # keep where freeidx >= partition
                out=ut128[:], in_=ut128[:], compare_op=AO.is_ge, fill=0.0,
                base=0, pattern=[[1, 128]], channel_multiplier=-1,
            )
            ones_col = cpool.tile([128, 1], f32)
            nc.vector.memset(ones_col[:], 1.0)
            ones_1x128 = cpool.tile([1, 128], f32)
            nc.vector.memset(ones_1x128[:], 1.0)
            ones_8 = cpool.tile([128, 8], f32)
            nc.vector.memset(ones_8[:], 1.0)
            ones_row256 = cpool.tile([1, 256], f32)
            nc.vector.memset(ones_row256[:], 1.0)
            eid = cpool.tile([128, 1], f32)
            nc.sync.dma_start(out=eid[:], in_=eidt[:])
            # expert index per free-slot: [128, 32, 8] values 0..7
            eidx_i = cpool.tile([128, 256], i32)
            nc.gpsimd.iota(eidx_i[:], pattern=[[0, 32], [1, 8]], base=0,
                           channel_multiplier=0)
            eidx = cpool.tile([128, 256], f32)
            nc.vector.tensor_copy(eidx[:], eidx_i[:])
            # token id per (p, b): t = b*128 + p, replicated x16 for scatter rows
            tiota_i = cpool.tile([128, 32, 16], i32)
            nc.gpsimd.iota(tiota_i[:], pattern=[[128, 32], [0, 16]], base=0,
                           channel_multiplier=1)
            # wg chunks [128, 16, 8]
            wgh = cpool.tile([128, DK, E], bf16)
            nc.sync.dma_start(out=wgh[:], in_=wg_hi.rearrange("(k p) e -> p k e", p=128))
            wgl = cpool.tile([128, DK, E], bf16)
            nc.sync.dma_start(out=wgl[:], in_=wg_lo.rearrange("(k p) e -> p k e", p=128))

            # ================= PHASE R: routing =================
            lg = rp.tile([128, 256], f32)  # logits token-major [128, b=32, e=8]
            with (
                tc.tile_pool(name="lgp", bufs=2, space="PSUM") as lgp,
                tc.tile_pool(name="ltp", bufs=2, space="PSUM") as ltp,
                tc.tile_pool(name="lsb", bufs=2) as lsb,
                tc.tile_pool(name="xts", bufs=4) as xts,
            ):
                for g in range(8):
                    ps_lg = lgp.tile([8, 512], f32)
                    for dk in range(DK):
                        xh = xts.tile([128, 512], bf16, tag="xh")
                        nc.sync.dma_start(out=xh[:], in_=xt_hi[dk, g])
                        xl = xts.tile([128, 512], bf16, tag="xl")
                        nc.sync.dma_start(out=xl[:], in_=xt_lo[dk, g])
                        wh_s = wgh[:, dk, :]
                        wl_s = wgl[:, dk, :]
                        nc.tensor.matmul(ps_lg[:], wh_s, xh[:], start=(dk == 0), stop=False)
                        nc.tensor.matmul(ps_lg[:], wl_s, xh[:], start=False, stop=False)
                        nc.tensor.matmul(ps_lg[:], wh_s, xl[:], start=False,
                                         stop=(dk == DK - 1))
                    lg_sb = lsb.tile([8, 512], f32)
                    nc.scalar.copy(lg_sb[:], ps_lg[:])
                    for q in range(4):
                        ps_t = ltp.tile([128, 8], f32)
                        nc.tensor.transpose(ps_t[:], lg_sb[:, q * 128:(q + 1) * 128],
                                            ident32[0:8, 0:8])
                        b = 4 * g + q
                        nc.vector.tensor_copy(lg[:, b * 8:(b + 1) * 8], ps_t[:])

            lg3 = lg[:].rearrange("p (b e) -> p b e", e=8)
            # --- top-2 on raw logits ---
            m1 = rp.tile([128, 32], f32)
            nc.vector.tensor_reduce(m1[:], lg3, axis=mybir.AxisListType.X, op=AO.max)
            oh0 = rp.tile([128, 256], f32)
            oh03 = oh0[:].rearrange("p (b e) -> p b e", e=8)
            for b in range(NB):
                nc.vector.tensor_scalar(oh03[:, b, :], lg3[:, b, :], m1[:, b:b + 1],
                                        None, AO.is_equal)
            # tie-break: keep only first match per token
            ohs = rp.tile([128, 256], f32)
            ohs3 = ohs[:].rearrange("p (b e) -> p b e", e=8)
            for b in range(NB):
                nc.vector.tensor_tensor_scan(ohs3[:, b, :], ones_8[:], oh03[:, b, :],
                                             0.0, AO.mult, AO.add)
            tmp = rp.tile([128, 256], f32)
            nc.vector.tensor_scalar(tmp[:], ohs[:], 1.0, None, AO.is_equal)
            nc.vector.tensor_mul(oh0[:], oh0[:], tmp[:])
            # masked logits -> m2, oh1
            lgm = rp.tile([128, 256], f32)
            nc.vector.scalar_tensor_tensor(lgm[:], oh0[:], -1e30, lg[:], AO.mult, AO.add)
            lgm3 = lgm[:].rearrange("p (b e) -> p b e", e=8)
            m2 = rp.tile([128, 32], f32)
            nc.vector.tensor_reduce(m2[:], lgm3, axis=mybir.AxisListType.X, op=AO.max)
            oh1 = rp.tile([128, 256], f32)
            oh13 = oh1[:].rearrange("p (b e) -> p b e", e=8)
            for b in range(NB):
                nc.vector.tensor_scalar(oh13[:, b, :], lgm3[:, b, :], m2[:, b:b + 1],
                                        None, AO.is_equal)
            for b in range(NB):
                nc.vector.tensor_tensor_scan(ohs3[:, b, :], ones_8[:], oh13[:, b, :],
                                             0.0, AO.mult, AO.add)
            nc.vector.tensor_scalar(tmp[:], ohs[:], 1.0, None, AO.is_equal)
            nc.vector.tensor_mul(oh1[:], oh1[:], tmp[:])

            # --- softmax pieces (max-subtracted) ---
            ex = rp.tile([128, 256], f32)
            ex3 = ex[:].rearrange("p (b e) -> p b e", e=8)
            for b in range(NB):
                nc.vector.tensor_scalar(ex3[:, b, :], lg3[:, b, :], m1[:, b:b + 1],
                                        None, AO.subtract)
            nc.scalar.activation(ex[:], ex[:], ACT.Exp)
            sumex = rp.tile([128, 32], f32)
            nc.vector.tensor_reduce(sumex[:], ex3, axis=mybir.AxisListType.X, op=AO.add)
            rec = rp.tile([128, 32], f32)
            nc.vector.reciprocal(rec[:], sumex[:])
            # gates: g0 = 1*rec ; g1 = exp(m2-m1)*rec
            d21 = rp.tile([128, 32], f32)
            nc.vector.tensor_sub(d21[:], m2[:], m1[:])
            nc.scalar.activation(d21[:], d21[:], ACT.Exp)
            g1t = rp.tile([128, 32], f32)
            nc.vector.tensor_mul(g1t[:], d21[:], rec[:])
            # normalized scores (for l_aux's me)
            S = rp.tile([128, 256], f32)
            S3 = S[:].rearrange("p (b e) -> p b e", e=8)
            for b in range(NB):
                nc.vector.tensor_scalar(S3[:, b, :], ex3[:, b, :], rec[:, b:b + 1],
                                        None, AO.mult)
            # expert indices of top1/top2
            idx0 = rp.tile([128, 32], f32)
            nc.vector.tensor_mul(tmp[:], oh0[:], eidx[:])
            nc.vector.tensor_reduce(idx0[:], tmp[:].rearrange("p (b e) -> p b e", e=8),
                                    axis=mybir.AxisListType.X, op=AO.add)
            idx1 = rp.tile([128, 32], f32)
            nc.vector.tensor_mul(tmp[:], oh1[:], eidx[:])
            nc.vector.tensor_reduce(idx1[:], tmp[:].rearrange("p (b e) -> p b e", e=8),
                                    axis=mybir.AxisListType.X, op=AO.add)

            # --- cumsum over token order (t = b*128 + p) ---
            with (
                tc.tile_pool(name="csp", bufs=2, space="PSUM") as cspp,
                tc.tile_pool(name="cump", bufs=2, space="PSUM") as cumpp,
            ):
                # per-(b, e) column sums  [1, 256]
                ps_cs0 = cspp.tile([1, 256], f32)
                nc.tensor.matmul(ps_cs0[:], ones_col[:], oh0[:], start=True, stop=True)
                css0 = rp.tile([1, 256], f32)
                nc.vector.tensor_copy(css0[:], ps_cs0[:])
                ps_cs1 = cspp.tile([1, 256], f32)
                nc.tensor.matmul(ps_cs1[:], ones_col[:], oh1[:], start=True, stop=True)
                css1 = rp.tile([1, 256], f32)
                nc.vector.tensor_copy(css1[:], ps_cs1[:])
                # inclusive prefix over b per e (scan over strided [1, 32] views)
                sc0 = rp.tile([1, 256], f32)
                sc03 = sc0[:].rearrange("o (b e) -> o e b", e=8)
                css03 = css0[:].rearrange("o (b e) -> o e b", e=8)
                for e in range(E):
                    nc.vector.tensor_tensor_scan(sc03[:, e, :], ones_row256[:, 0:32],
                                                 css03[:, e, :], 0.0, AO.mult, AO.add)
                sc1 = rp.tile([1, 256], f32)
                sc13 = sc1[:].rearrange("o (b e) -> o e b", e=8)
                css13 = css1[:].rearrange("o (b e) -> o e b", e=8)
                for e in range(E):
                    nc.vector.tensor_tensor_scan(sc13[:, e, :], ones_row256[:, 0:32],
                                                 css13[:, e, :], 0.0, AO.mult, AO.add)
                # exclusive prefixes
                pre0 = rp.tile([1, 256], f32)
                nc.vector.tensor_sub(pre0[:], sc0[:], css0[:])
                pre1 = rp.tile([1, 256], f32)
                nc.vector.tensor_sub(pre1[:], sc1[:], css1[:])
                # slot-1 positions come after ALL slot-0 assignments:
                # add total0[e] (= sc0 at b=31) broadcast over b
                tot0 = sc0[0:1, 248:256]  # [1, 8]
                pre1b = pre1[0:1, :].rearrange("o (b e) -> o b e", e=8)
                nc.vector.tensor_tensor(pre1b, pre1b,
                                        tot0.rearrange("o (u e) -> o u e", u=1)
                                        .to_broadcast([1, 32, 8]),
                                        AO.add)
                # cum (inclusive within-expert position + 1) = UT@oh + bcast(prefix)
                cum0 = rp.tile([128, 256], f32)
                ps_c0 = cumpp.tile([128, 256], f32)
                nc.tensor.matmul(ps_c0[:], ut128[:], oh0[:], start=True, stop=False)
                nc.tensor.matmul(ps_c0[:], ones_1x128[:], pre0[:], start=False, stop=True)
                nc.vector.tensor_copy(cum0[:], ps_c0[:])
                cum1 = rp.tile([128, 256], f32)
                ps_c1 = cumpp.tile([128, 256], f32)
                nc.tensor.matmul(ps_c1[:], ut128[:], oh1[:], start=True, stop=False)
                nc.tensor.matmul(ps_c1[:], ones_1x128[:], pre1[:], start=False, stop=True)
                nc.vector.tensor_copy(cum1[:], ps_c1[:])

            # per-token slot position within its expert (0-based)
            c0 = rp.tile([128, 32], f32)
            nc.vector.tensor_mul(tmp[:], cum0[:], oh0[:])
            nc.vector.tensor_reduce(c0[:], tmp[:].rearrange("p (b e) -> p b e", e=8),
                                    axis=mybir.AxisListType.X, op=AO.add)
            nc.vector.tensor_scalar(c0[:], c0[:], 1.0, None, AO.subtract)
            c1 = rp.tile([128, 32], f32)
            nc.vector.tensor_mul(tmp[:], cum1[:], oh1[:])
            nc.vector.tensor_reduce(c1[:], tmp[:].rearrange("p (b e) -> p b e", e=8),
                                    axis=mybir.AxisListType.X, op=AO.add)
            nc.vector.tensor_scalar(c1[:], c1[:], 1.0, None, AO.subtract)

            # --- per-core (expert e = eid) selection ---
            sel0 = rp.tile([128, 32], f32)
            nc.vector.tensor_scalar(sel0[:], idx0[:], eid[:, 0:1], None, AO.is_equal)
            sel1 = rp.tile([128, 32], f32)
            nc.vector.tensor_scalar(sel1[:], idx1[:], eid[:, 0:1], None, AO.is_equal)
            kept0 = rp.tile([128, 32], f32)
            nc.vector.tensor_scalar(kept0[:], c0[:], float(C_PAD), None, AO.is_lt)
            nc.vector.tensor_mul(kept0[:], kept0[:], sel0[:])
            kept1 = rp.tile([128, 32], f32)
            nc.vector.tensor_scalar(kept1[:], c1[:], float(C_PAD), None, AO.is_lt)
            nc.vector.tensor_mul(kept1[:], kept1[:], sel1[:])
            # coeff = kept0*g0 + kept1*g1   (g0 = rec)
            coeff = pp.tile([128, 32], f32)
            nc.vector.tensor_mul(coeff[:], kept0[:], rec[:])
            t2 = rp.tile([128, 32], f32)
            nc.vector.tensor_mul(t2[:], kept1[:], g1t[:])
            nc.vector.tensor_add(coeff[:], coeff[:], t2[:])
            # slotof = kept0*c0 + kept1*c1 + (1-kept0-kept1)*C_PAD
            slotof = rp.tile([128, 32], f32)
            nc.vector.tensor_mul(slotof[:], kept0[:], c0[:])
            nc.vector.tensor_mul(t2[:], kept1[:], c1[:])
            nc.vector.tensor_add(slotof[:], slotof[:], t2[:])
            kk = rp.tile([128, 32], f32)
            nc.vector.tensor_add(kk[:], kept0[:], kept1[:])
            nc.vector.scalar_tensor_tensor(slotof[:], kk[:], -float(C_PAD), slotof[:],
                                           AO.mult, AO.add)
            nc.vector.tensor_scalar(slotof[:], slotof[:], float(C_PAD), None, AO.add)
            slot_i = pp.tile([128, 32], i32)
            nc.vector.tensor_copy(slot_i[:], slotof[:])

            # --- l_aux ---
            with tc.tile_pool(name="lap", bufs=1, space="PSUM") as lap:
                ps_me = lap.tile([1, 256], f32)
                nc.tensor.matmul(ps_me[:], ones_col[:], S[:], start=True, stop=True)
                me_b = rp.tile([1, 256], f32)
                nc.vector.tensor_copy(me_b[:], ps_me[:])
            me8 = rp.tile([1, 8], f32)
            nc.vector.tensor_reduce(me8[:], me_b[:].rearrange("o (b e) -> o e b", e=8),
                                    axis=mybir.AxisListType.X, op=AO.add)
            ce8 = rp.tile([1, 8], f32)
            nc.vector.tensor_reduce(ce8[:], css0[:].rearrange("o (b e) -> o e b", e=8),
                                    axis=mybir.AxisListType.X, op=AO.add)
            nc.vector.tensor_mul(me8[:], me8[:], ce8[:])
            la = rp.tile([1, 1], f32)
            nc.vector.tensor_reduce(la[:], me8[:], axis=mybir.AxisListType.X, op=AO.add)
            nc.vector.tensor_scalar(la[:], la[:], float(E) / (T * T), None, AO.mult)
            nc.sync.dma_start(out=laux[:], in_=la[:])

            # --- tokmap scatter: tokmapd[slot] = token id ---
            zrow = rp.tile([128, 11, 16], i32)
            nc.vector.memset(zrow[:], 0)
            nc.sync.dma_start(
                out=tokmapd.rearrange("(ci p) w -> p ci w", p=128), in_=zrow[:])
            for g in range(NB):
                nc.gpsimd.indirect_dma_start(
                    out=tokmapd[:], out_offset=bass.IndirectOffsetOnAxis(
                        ap=slot_i[:, g:g + 1], axis=0),
                    in_=tiota_i[:, g, :], in_offset=None,
                )

            # ================= PHASE E: encode (gather + transpose) ===========
            dispT = pp.tile([128, DK * C_PAD], f16)  # [d-chunk][128d, 1280c]
            tokmap_sb = pp.tile([128, NC_CHUNKS], i32)
            nc.sync.dma_start(
                out=tokmap_sb[:],
                in_=tokmapd.rearrange("(ci p) w -> p ci w", p=128)[:, 0:NC_CHUNKS, 0])
            with (
                tc.tile_pool(name="disp", bufs=3) as dp,
                tc.tile_pool(name="trp", bufs=4, space="PSUM") as trp,
            ):
                for ci in range(NC_CHUNKS):
                    dchunk = dp.tile([128, D], f16, tag="dchunk")
                    nc.gpsimd.indirect_dma_start(
                        out=dchunk[:], out_offset=None,
                        in_=x16[:], in_offset=bass.IndirectOffsetOnAxis(
                            ap=tokmap_sb[:, ci:ci + 1], axis=0),
                    )
                    for dk in range(DK):
                        ps_tr = trp.tile([128, 128], f16)
                        nc.tensor.transpose(ps_tr[:], dchunk[:, dk * 128:(dk + 1) * 128],
                                            ident16[:])
                        nc.vector.tensor_copy(
                            dispT[:, dk * C_PAD + ci * 128: dk * C_PAD + (ci + 1) * 128],
                            ps_tr[:])

            # ================= PHASE F: FFN =================
            b1c = cpool.tile([128, HQ], f32)
            nc.sync.dma_start(out=b1c[:], in_=b1v.rearrange("(hq p) -> p hq", p=128))
            b2row = cpool.tile([1, D], f32)
            nc.sync.dma_start(out=b2row[:], in_=b2v.rearrange("(u d) -> u d", u=1))
            zero16 = cpool.tile([1, D], f16)
            nc.vector.memset(zero16[:], 0.0)
            nc.sync.dma_start(out=y_dram[C_PAD:C_PAD + 1, :], in_=zero16[:])

            c_sups = [(0, 512), (512, 512), (1024, 256)]
            with (
                tc.tile_pool(name="hT", bufs=1) as hpool,
                tc.tile_pool(name="w1s", bufs=24) as w1p,
                tc.tile_pool(name="w2s", bufs=6) as w2p,
                tc.tile_pool(name="psh", bufs=1, space="PSUM") as pshp,
                tc.tile_pool(name="psy", bufs=1, space="PSUM") as psyp,
                tc.tile_pool(name="ysb", bufs=4) as yp,
            ):
                hT = hpool.tile([128, HQ * 512], f16)
                for cs, cw in c_sups:
                    # FFN1: h_T[h', c] = relu(sum_d w1[d, h'] * dispT[d, c] + b1)
                    for hqb in range(HQ // 4):  # blocks of 4 h'-chunks
                        ps_hs = [pshp.tile([128, 512], f32, tag=f"ps_h{j}",
                                           name=f"ps_h{j}")
                                 for j in range(4)]
                        for dk in range(DK):
                            w1t = w1p.tile([128, 512], f16, tag="w1t")
                            nc.sync.dma_start(out=w1t[:], in_=w1c[dk, hqb])
                            for j in range(4):
                                nc.tensor.matmul(
                                    ps_hs[j][:, 0:cw],
                                    w1t[:, j * 128:(j + 1) * 128],
                                    dispT[:, dk * C_PAD + cs: dk * C_PAD + cs + cw],
                                    start=(dk == 0), stop=(dk == DK - 1))
                        for j in range(4):
                            hq = hqb * 4 + j
                            nc.scalar.activation(hT[:, hq * 512: hq * 512 + cw],
                                                 ps_hs[j][:, 0:cw], ACT.Relu,
                                                 bias=b1c[:, hq:hq + 1])
                    # FFN2: y[c, d] = sum_h' h_T[h', c] * w2[h', d] + b2
                    ncc = cw // 128
                    for dg in range(4):
                        ps_ys = [psyp.tile([128, 512], f32, tag=f"ps_y{cc}",
                                           name=f"ps_y{cc}")
                                 for cc in range(ncc)]
                        for hk in range(HQ):
                            w2t = w2p.tile([128, 512], f16, tag="w2t")
                            nc.scalar.dma_start(out=w2t[:], in_=w2c[hk, dg])
                            for cc in range(ncc):
                                nc.tensor.matmul(
                                    ps_ys[cc][:],
                                    hT[:, hk * 512 + cc * 128: hk * 512 + (cc + 1) * 128],
                                    w2t[:],
                                    start=(hk == 0), stop=False)
                        for cc in range(ncc):
                            nc.tensor.matmul(ps_ys[cc][:], ones_1x128[:],
                                             b2row[:, dg * 512:(dg + 1) * 512],
                                             start=False, stop=True)
                            y_sb = yp.tile([128, 512], f16, tag="y_sb")
                            nc.vector.tensor_copy(y_sb[:], ps_ys[cc][:])
                            nc.sync.dma_start(
                                out=y_dram[cs + cc * 128: cs + (cc + 1) * 128,
                                           dg * 512:(dg + 1) * 512],
                                in_=y_sb[:])

            # ========= PHASE D: decode + chunked AllReduce overlap =========
            AR_CHUNK = 8  # token-blocks per AllReduce slab (8*128 rows = 8 MB)
            with (
                tc.tile_pool(name="yt", bufs=3) as ytp,
                tc.tile_pool(name="op", bufs=3) as opp,
            ):
                for g in range(NB):
                    yt = ytp.tile([128, D], f16, tag="yt")
                    nc.gpsimd.indirect_dma_start(
                        out=yt[:], out_offset=None,
                        in_=y_dram[:], in_offset=bass.IndirectOffsetOnAxis(
                            ap=slot_i[:, g:g + 1], axis=0),
                    )
                    op_t = opp.tile([128, D], f32, tag="op_t")
                    nc.vector.tensor_scalar(op_t[:], yt[:], coeff[:, g:g + 1],
                                            None, AO.mult)
                    nc.sync.dma_start(out=partial_d[g * 128:(g + 1) * 128, :],
                                      in_=op_t[:])
                    if (g + 1) % AR_CHUNK == 0:
                        r0, r1 = (g + 1 - AR_CHUNK) * 128, (g + 1) * 128
                        nc.gpsimd.collective_compute(
                            "AllReduce", AO.add,
                            replica_groups=[list(range(N_CORES))],
                            ins=[partial_d[r0:r1, :]], outs=[ar_out[r0:r1, :]],
                        )
                        nc.sync.dma_start(out=out[r0:r1, :], in_=ar_out[r0:r1, :])

    nc.compile()
    _BUILD_CACHE["nc"] = nc
    return nc


def _prep_inputs(x, wg, w1, b1, w2, b2):
    x = np.asarray(x, np.float32)
    wg = np.asarray(wg, np.float32)
    w1 = np.asarray(w1, np.float32)
    b1 = np.asarray(b1, np.float32)
    w2 = np.asarray(w2, np.float32)
    b2 = np.asarray(b2, np.float32)

    xT = np.ascontiguousarray(x.T)  # [D, T]
    xt_hi = xT.astype(ml_dtypes.bfloat16)
    xt_lo = (xT - xt_hi.astype(np.float32)).astype(ml_dtypes.bfloat16)
    # chunk layout [dk, g, 128, 512]
    xt_hi = np.ascontiguousarray(
        xt_hi.reshape(16, 128, 8, 512).transpose(0, 2, 1, 3))
    xt_lo = np.ascontiguousarray(
        xt_lo.reshape(16, 128, 8, 512).transpose(0, 2, 1, 3))
    wg_hi = wg.astype(ml_dtypes.bfloat16)
    wg_lo = (wg - wg_hi.astype(np.float32)).astype(ml_dtypes.bfloat16)
    x16 = x.astype(np.float16)

    shared = {
        "xt_hi": xt_hi, "xt_lo": xt_lo,
        "wg_hi": wg_hi, "wg_lo": wg_lo, "x16": x16,
    }
    in_maps = []
    for e in range(N_CORES):
        w1e = w1[e].astype(np.float16)  # [D, H]
        w1ce = np.ascontiguousarray(
            w1e.reshape(16, 128, 16, 512).transpose(0, 2, 1, 3))
        w2e = w2[e].astype(np.float16)  # [H, D]
        w2ce = np.ascontiguousarray(
            w2e.reshape(64, 128, 4, 512).transpose(0, 2, 1, 3))
        m = dict(shared)
        m["w1c"] = w1ce
        m["w2c"] = w2ce
        m["b1v"] = np.ascontiguousarray(b1[e, 0])
        m["b2v"] = np.ascontiguousarray(b2[e, 0])
        m["eidt"] = np.full((128, 1), float(e), np.float32)
        in_maps.append(m)
    return in_maps


def kernel(x, wg, w1, b1, w2, b2, k=2, _want_results=False, _trace=False, **_ignored):
    assert int(k) == K
    nc = build_bass()
    in_maps = _prep_inputs(x, wg, w1, b1, w2, b2)
    res = run_bass_kernel_spmd(nc, in_maps, core_ids=list(range(N_CORES)),
                               trace=_trace)
    out = np.asarray(res.results[0]["out"])
    laux = np.float32(np.asarray(res.results[0]["laux"])[0])
    if _want_results:
        return (out, laux), res
    return out, laux


# revision 20
# speedup vs baseline: 1.6773x; 1.1030x over previous
"""Trainium2 Bass kernel for CustomMoE (top-2 routing, E=8 experts, expert parallel).

Contract: kernel(**inputs) takes FULL unsharded inputs (x, wg, w1, b1, w2, b2, k)
and returns (output [T, D] fp32, l_aux scalar fp32) matching reference().

Strategy (8 NeuronCores, expert parallelism — one expert per core):
  - routing replicated on every core (cheap), computed in token-major layout
  - each core gathers the tokens routed to its expert into capacity slots
    (C_PAD=1280 slots; mean load is 1024, so 8.5-sigma headroom; tokens are
    dropped exactly like the reference iff any load exceeded C_PAD, which is
    unreachable for randn-style inputs), runs the FFN on its expert weights
    in fp16 (fp22 accumulate), scatters gate-scaled results back per token,
  - AllReduce over the 8 cores combines the per-expert partial outputs.
"""

import numpy as np
import ml_dtypes

import concourse.bass as bass
import concourse.bacc as bacc
import concourse.mybir as mybir
import concourse.tile as tile
from concourse.bass_utils import run_bass_kernel_spmd
from concourse.masks import make_identity

f32 = mybir.dt.float32
f16 = mybir.dt.float16
bf16 = mybir.dt.bfloat16
i32 = mybir.dt.int32
AO = mybir.AluOpType
ACT = mybir.ActivationFunctionType

T, D, H, E, K = 4096, 2048, 8192, 8, 2
NB = T // 128            # 32 token blocks (token t = b*128 + p)
C_PAD = 1280             # capacity slots materialized per expert (>= max load)
NC_CHUNKS = C_PAD // 128  # 10
DK = D // 128            # 16 contraction chunks over D
HQ = H // 128            # 64 chunks over H
N_CORES = 8

_BUILD_CACHE = {}


def build_bass():
    if "nc" in _BUILD_CACHE:
        return _BUILD_CACHE["nc"]
    nc = bacc.Bacc(None)

    # ---- inputs (per core; routing inputs identical on all cores) ----
    xt_hi = nc.dram_tensor("xt_hi", [16, 8, 128, 512], bf16, kind="ExternalInput")
    xt_lo = nc.dram_tensor("xt_lo", [16, 8, 128, 512], bf16, kind="ExternalInput")
    wg_hi = nc.dram_tensor("wg_hi", [D, E], bf16, kind="ExternalInput")
    wg_lo = nc.dram_tensor("wg_lo", [D, E], bf16, kind="ExternalInput")
    x16 = nc.dram_tensor("x16", [T, D], f16, kind="ExternalInput")
    w1c = nc.dram_tensor("w1c", [DK, HQ // 4, 128, 512], f16, kind="ExternalInput")
    w2c = nc.dram_tensor("w2c", [HQ, 4, 128, 512], f16, kind="ExternalInput")
    b1v = nc.dram_tensor("b1v", [H], f32, kind="ExternalInput")
    b2v = nc.dram_tensor("b2v", [D], f32, kind="ExternalInput")
    eidt = nc.dram_tensor("eidt", [128, 1], f32, kind="ExternalInput")

    # ---- outputs ----
    out = nc.dram_tensor("out", [T, D], f32, kind="ExternalOutput")
    laux = nc.dram_tensor("laux", [1], f32, kind="ExternalOutput")

    # ---- internal DRAM ----
    tokmapd = nc.dram_tensor("tokmapd", [C_PAD + 128, 16], i32)
    y_dram = nc.dram_tensor("y_dram", [C_PAD + 1, D], f16)
    partial_d = nc.dram_tensor("partial_d", [T, D], f16)
    ar_out = nc.dram_tensor("ar_out", [T, D], f16, addr_space="Shared")

    with tile.TileContext(nc) as tc:
        with (
            tc.tile_pool(name="const", bufs=1) as cpool,
            tc.tile_pool(name="rout", bufs=1) as rp,
            tc.tile_pool(name="persist", bufs=1) as pp,
        ):
            # ================= constants =================
            ident16 = cpool.tile([128, 128], f16)
            make_identity(nc, ident16[:])
            ident32 = cpool.tile([128, 128], f32)
            make_identity(nc, ident32[:])
            ut128 = cpool.tile([128, 128], f32)
            nc.vector.memset(ut128[:], 1.0)
            nc.gpsimd.affine_select(  # keep where freeidx >= partition
                out=ut128[:], in_=ut128[:], compare_op=AO.is_ge, fill=0.0,
                base=0, pattern=[[1, 128]], channel_multiplier=-1,
            )
            ones_col = cpool.tile([128, 1], f32)
            nc.vector.memset(ones_col[:], 1.0)
            ones_1x128 = cpool.tile([1, 128], f32)
            nc.vector.memset(ones_1x128[:], 1.0)
            ones_8 = cpool.tile([128, 8], f32)
            nc.vector.memset(ones_8[:], 1.0)
            ones_row256 = cpool.tile([1, 256], f32)
            nc.vector.memset(ones_row256[:], 1.0)
            eid = cpool.tile([128, 1], f32)
            nc.sync.dma_start(out=eid[:], in_=eidt[:])
            # expert index per free-slot: [128, 32, 8] values 0..7
            eidx_i = cpool.tile([128, 256], i32)
            nc.gpsimd.iota(eidx_i[:], pattern=[[0, 32], [1, 8]], base=0,
                           channel_multiplier=0)
            eidx = cpool.tile([128, 256], f32)
            nc.vector.tensor_copy(eidx[:], eidx_i[:])
            # token id per (p, b): t = b*128 + p, replicated x16 for scatter rows
            tiota_i = cpool.tile([128, 32, 16], i32)
            nc.gpsimd.iota(tiota_i[:], pattern=[[128, 32], [0, 16]], base=0,
                           channel_multiplier=1)
            # wg chunks [128, 16, 8]
            wgh = cpool.tile([128, DK, E], bf16)
            nc.sync.dma_start(out=wgh[:], in_=wg_hi.rearrange("(k p) e -> p k e", p=128))
            wgl = cpool.tile([128, DK, E], bf16)
            nc.sync.dma_start(out=wgl[:], in_=wg_lo.rearrange("(k p) e -> p k e", p=128))

            # ================= PHASE R: routing =================
            lg = rp.tile([128, 256], f32)  # logits token-major [128, b=32, e=8]
            with (
                tc.tile_pool(name="lgp", bufs=2, space="PSUM") as lgp,
                tc.tile_pool(name="ltp", bufs=2, space="PSUM") as ltp,
                tc.tile_pool(name="lsb", bufs=2) as lsb,
                tc.tile_pool(name="xts", bufs=4) as xts,
            ):
                for g in range(8):
                    ps_lg = lgp.tile([8, 512], f32)
                    for dk in range(DK):
                        xh = xts.tile([128, 512], bf16, tag="xh")
                        nc.sync.dma_start(out=xh[:], in_=xt_hi[dk, g])
                        xl = xts.tile([128, 512], bf16, tag="xl")
                        nc.sync.dma_start(out=xl[:], in_=xt_lo[dk, g])
                        wh_s = wgh[:, dk, :]
                        wl_s = wgl[:, dk, :]
                        nc.tensor.matmul(ps_lg[:], wh_s, xh[:], start=(dk == 0), stop=False)
                        nc.tensor.matmul(ps_lg[:], wl_s, xh[:], start=False, stop=False)
                        nc.tensor.matmul(ps_lg[:], wh_s, xl[:], start=False,
                                         stop=(dk == DK - 1))
                    lg_sb = lsb.tile([8, 512], f32)
                    nc.scalar.copy(lg_sb[:], ps_lg[:])
                    for q in range(4):
                        ps_t = ltp.tile([128, 8], f32)
                        nc.tensor.transpose(ps_t[:], lg_sb[:, q * 128:(q + 1) * 128],
                                            ident32[0:8, 0:8])
                        b = 4 * g + q
                        nc.vector.tensor_copy(lg[:, b * 8:(b + 1) * 8], ps_t[:])

            lg3 = lg[:].rearrange("p (b e) -> p b e", e=8)
            # --- top-2 on raw logits ---
            m1 = rp.tile([128, 32], f32)
            nc.vector.tensor_reduce(m1[:], lg3, axis=mybir.AxisListType.X, op=AO.max)
            oh0 = rp.tile([128, 256], f32)
            oh03 = oh0[:].rearrange("p (b e) -> p b e", e=8)
            for b in range(NB):
                nc.vector.tensor_scalar(oh03[:, b, :], lg3[:, b, :], m1[:, b:b + 1],
                                        None, AO.is_equal)
            # tie-break: keep only first match per token
            ohs = rp.tile([128, 256], f32)
            ohs3 = ohs[:].rearrange("p (b e) -> p b e", e=8)
            for b in range(NB):
                nc.vector.tensor_tensor_scan(ohs3[:, b, :], ones_8[:], oh03[:, b, :],
                                             0.0, AO.mult, AO.add)
            tmp = rp.tile([128, 256], f32)
            nc.vector.tensor_scalar(tmp[:], ohs[:], 1.0, None, AO.is_equal)
            nc.vector.tensor_mul(oh0[:], oh0[:], tmp[:])
            # masked logits -> m2, oh1
            lgm = rp.tile([128, 256], f32)
            nc.vector.scalar_tensor_tensor(lgm[:], oh0[:], -1e30, lg[:], AO.mult, AO.add)
            lgm3 = lgm[:].rearrange("p (b e) -> p b e", e=8)
            m2 = rp.tile([128, 32], f32)
            nc.vector.tensor_reduce(m2[:], lgm3, axis=mybir.AxisListType.X, op=AO.max)
            oh1 = rp.tile([128, 256], f32)
            oh13 = oh1[:].rearrange("p (b e) -> p b e", e=8)
            for b in range(NB):
                nc.vector.tensor_scalar(oh13[:, b, :], lgm3[:, b, :], m2[:, b:b + 1],
                                        None, AO.is_equal)
            for b in range(NB):
                nc.vector.tensor_tensor_scan(ohs3[:, b, :], ones_8[:], oh13[:, b, :],
                                             0.0, AO.mult, AO.add)
            nc.vector.tensor_scalar(tmp[:], ohs[:], 1.0, None, AO.is_equal)
            nc.vector.tensor_mul(oh1[:], oh1[:], tmp[:])

            # --- softmax pieces (max-subtracted) ---
            ex = rp.tile([128, 256], f32)
            ex3 = ex[:].rearrange("p (b e) -> p b e", e=8)
            for b in range(NB):
                nc.vector.tensor_scalar(ex3[:, b, :], lg3[:, b, :], m1[:, b:b + 1],
                                        None, AO.subtract)
            nc.scalar.activation(ex[:], ex[:], ACT.Exp)
            sumex = rp.tile([128, 32], f32)
            nc.vector.tensor_reduce(sumex[:], ex3, axis=mybir.AxisListType.X, op=AO.add)
            rec = rp.tile([128, 32], f32)
            nc.vector.reciprocal(rec[:], sumex[:])
            # gates: g0 = 1*rec ; g1 = exp(m2-m1)*rec
            d21 = rp.tile([128, 32], f32)
            nc.vector.tensor_sub(d21[:], m2[:], m1[:])
            nc.scalar.activation(d21[:], d21[:], ACT.Exp)
            g1t = rp.tile([128, 32], f32)
            nc.vector.tensor_mul(g1t[:], d21[:], rec[:])
            # normalized scores (for l_aux's me)
            S = rp.tile([128, 256], f32)
            S3 = S[:].rearrange("p (b e) -> p b e", e=8)
            for b in range(NB):
                nc.vector.tensor_scalar(S3[:, b, :], ex3[:, b, :], rec[:, b:b + 1],
                                        None, AO.mult)
            # expert indices of top1/top2
            idx0 = rp.tile([128, 32], f32)
            nc.vector.tensor_mul(tmp[:], oh0[:], eidx[:])
            nc.vector.tensor_reduce(idx0[:], tmp[:].rearrange("p (b e) -> p b e", e=8),
                                    axis=mybir.AxisListType.X, op=AO.add)
            idx1 = rp.tile([128, 32], f32)
            nc.vector.tensor_mul(tmp[:], oh1[:], eidx[:])
            nc.vector.tensor_reduce(idx1[:], tmp[:].rearrange("p (b e) -> p b e", e=8),
                                    axis=mybir.AxisListType.X, op=AO.add)

            # --- cumsum over token order (t = b*128 + p) ---
            with (
                tc.tile_pool(name="csp", bufs=2, space="PSUM") as cspp,
                tc.tile_pool(name="cump", bufs=2, space="PSUM") as cumpp,
            ):
                # per-(b, e) column sums  [1, 256]
                ps_cs0 = cspp.tile([1, 256], f32)
                nc.tensor.matmul(ps_cs0[:], ones_col[:], oh0[:], start=True, stop=True)
                css0 = rp.tile([1, 256], f32)
                nc.vector.tensor_copy(css0[:], ps_cs0[:])
                ps_cs1 = cspp.tile([1, 256], f32)
                nc.tensor.matmul(ps_cs1[:], ones_col[:], oh1[:], start=True, stop=True)
                css1 = rp.tile([1, 256], f32)
                nc.vector.tensor_copy(css1[:], ps_cs1[:])
                # inclusive prefix over b per e (scan over strided [1, 32] views)
                sc0 = rp.tile([1, 256], f32)
                sc03 = sc0[:].rearrange("o (b e) -> o e b", e=8)
                css03 = css0[:].rearrange("o (b e) -> o e b", e=8)
                for e in range(E):
                    nc.vector.tensor_tensor_scan(sc03[:, e, :], ones_row256[:, 0:32],
                                                 css03[:, e, :], 0.0, AO.mult, AO.add)
                sc1 = rp.tile([1, 256], f32)
                sc13 = sc1[:].rearrange("o (b e) -> o e b", e=8)
                css13 = css1[:].rearrange("o (b e) -> o e b", e=8)
                for e in range(E):
                    nc.vector.tensor_tensor_scan(sc13[:, e, :], ones_row256[:, 0:32],
                                                 css13[:, e, :], 0.0, AO.mult, AO.add)
                # exclusive prefixes
                pre0 = rp.tile([1, 256], f32)
                nc.vector.tensor_sub(pre0[:], sc0[:], css0[:])
                pre1 = rp.tile([1, 256], f32)
                nc.vector.tensor_sub(pre1[:], sc1[:], css1[:])
                # slot-1 positions come after ALL slot-0 assignments:
                # add total0[e] (= sc0 at b=31) broadcast over b
                tot0 = sc0[0:1, 248:256]  # [1, 8]
                pre1b = pre1[0:1, :].rearrange("o (b e) -> o b e", e=8)
                nc.vector.tensor_tensor(pre1b, pre1b,
                                        tot0.rearrange("o (u e) -> o u e", u=1)
                                        .to_broadcast([1, 32, 8]),
                                        AO.add)
                # cum (inclusive within-expert position + 1) = UT@oh + bcast(prefix)
                cum0 = rp.tile([128, 256], f32)
                ps_c0 = cumpp.tile([128, 256], f32)
                nc.tensor.matmul(ps_c0[:], ut128[:], oh0[:], start=True, stop=False)
                nc.tensor.matmul(ps_c0[:], ones_1x128[:], pre0[:], start=False, stop=True)
                nc.vector.tensor_copy(cum0[:], ps_c0[:])
                cum1 = rp.tile([128, 256], f32)
                ps_c1 = cumpp.tile([128, 256], f32)
                nc.tensor.matmul(ps_c1[:], ut128[:], oh1[:], start=True, stop=False)
                nc.tensor.matmul(ps_c1[:], ones_1x128[:], pre1[:], start=False, stop=True)
                nc.vector.tensor_copy(cum1[:], ps_c1[:])

            # per-token slot position within its expert (0-based)
            c0 = rp.tile([128, 32], f32)
            nc.vector.tensor_mul(tmp[:], cum0[:], oh0[:])
            nc.vector.tensor_reduce(c0[:], tmp[:].rearrange("p (b e) -> p b e", e=8),
                                    axis=mybir.AxisListType.X, op=AO.add)
            nc.vector.tensor_scalar(c0[:], c0[:], 1.0, None, AO.subtract)
            c1 = rp.tile([128, 32], f32)
            nc.vector.tensor_mul(tmp[:], cum1[:], oh1[:])
            nc.vector.tensor_reduce(c1[:], tmp[:].rearrange("p (b e) -> p b e", e=8),
                                    axis=mybir.AxisListType.X, op=AO.add)
            nc.vector.tensor_scalar(c1[:], c1[:], 1.0, None, AO.subtract)

            # --- per-core (expert e = eid) selection ---
            sel0 = rp.tile([128, 32], f32)
            nc.vector.tensor_scalar(sel0[:], idx0[:], eid[:, 0:1], None, AO.is_equal)
            sel1 = rp.tile([128, 32], f32)
            nc.vector.tensor_scalar(sel1[:], idx1[:], eid[:, 0:1], None, AO.is_equal)
            kept0 = rp.tile([128, 32], f32)
            nc.vector.tensor_scalar(kept0[:], c0[:], float(C_PAD), None, AO.is_lt)
            nc.vector.tensor_mul(kept0[:], kept0[:], sel0[:])
            kept1 = rp.tile([128, 32], f32)
            nc.vector.tensor_scalar(kept1[:], c1[:], float(C_PAD), None, AO.is_lt)
            nc.vector.tensor_mul(kept1[:], kept1[:], sel1[:])
            # coeff = kept0*g0 + kept1*g1   (g0 = rec)
            coeff = pp.tile([128, 32], f32)
            nc.vector.tensor_mul(coeff[:], kept0[:], rec[:])
            t2 = rp.tile([128, 32], f32)
            nc.vector.tensor_mul(t2[:], kept1[:], g1t[:])
            nc.vector.tensor_add(coeff[:], coeff[:], t2[:])
            # slotof = kept0*c0 + kept1*c1 + (1-kept0-kept1)*C_PAD
            slotof = rp.tile([128, 32], f32)
            nc.vector.tensor_mul(slotof[:], kept0[:], c0[:])
            nc.vector.tensor_mul(t2[:], kept1[:], c1[:])
            nc.vector.tensor_add(slotof[:], slotof[:], t2[:])
            kk = rp.tile([128, 32], f32)
            nc.vector.tensor_add(kk[:], kept0[:], kept1[:])
            nc.vector.scalar_tensor_tensor(slotof[:], kk[:], -float(C_PAD), slotof[:],
                                           AO.mult, AO.add)
            nc.vector.tensor_scalar(slotof[:], slotof[:], float(C_PAD), None, AO.add)
            slot_i = pp.tile([128, 32], i32)
            nc.vector.tensor_copy(slot_i[:], slotof[:])

            # --- l_aux ---
            with tc.tile_pool(name="lap", bufs=1, space="PSUM") as lap:
                ps_me = lap.tile([1, 256], f32)
                nc.tensor.matmul(ps_me[:], ones_col[:], S[:], start=True, stop=True)
                me_b = rp.tile([1, 256], f32)
                nc.vector.tensor_copy(me_b[:], ps_me[:])
            me8 = rp.tile([1, 8], f32)
            nc.vector.tensor_reduce(me8[:], me_b[:].rearrange("o (b e) -> o e b", e=8),
                                    axis=mybir.AxisListType.X, op=AO.add)
            ce8 = rp.tile([1, 8], f32)
            nc.vector.tensor_reduce(ce8[:], css0[:].rearrange("o (b e) -> o e b", e=8),
                                    axis=mybir.AxisListType.X, op=AO.add)
            nc.vector.tensor_mul(me8[:], me8[:], ce8[:])
            la = rp.tile([1, 1], f32)
            nc.vector.tensor_reduce(la[:], me8[:], axis=mybir.AxisListType.X, op=AO.add)
            nc.vector.tensor_scalar(la[:], la[:], float(E) / (T * T), None, AO.mult)
            nc.sync.dma_start(out=laux[:], in_=la[:])

            # --- tokmap scatter: tokmapd[slot] = token id ---
            zrow = rp.tile([128, 11, 16], i32)
            nc.vector.memset(zrow[:], 0)
            nc.sync.dma_start(
                out=tokmapd.rearrange("(ci p) w -> p ci w", p=128), in_=zrow[:])
            for g in range(NB):
                nc.gpsimd.indirect_dma_start(
                    out=tokmapd[:], out_offset=bass.IndirectOffsetOnAxis(
                        ap=slot_i[:, g:g + 1], axis=0),
                    in_=tiota_i[:, g, :], in_offset=None,
                )

            # ================= PHASE E: encode (gather + transpose) ===========
            dispT = pp.tile([128, DK * C_PAD], f16)  # [d-chunk][128d, 1280c]
            tokmap_sb = pp.tile([128, NC_CHUNKS], i32)
            nc.sync.dma_start(
                out=tokmap_sb[:],
                in_=tokmapd.rearrange("(ci p) w -> p ci w", p=128)[:, 0:NC_CHUNKS, 0])
            with (
                tc.tile_pool(name="disp", bufs=3) as dp,
                tc.tile_pool(name="trp", bufs=4, space="PSUM") as trp,
            ):
                for ci in range(NC_CHUNKS):
                    dchunk = dp.tile([128, D], f16, tag="dchunk")
                    nc.gpsimd.indirect_dma_start(
                        out=dchunk[:], out_offset=None,
                        in_=x16[:], in_offset=bass.IndirectOffsetOnAxis(
                            ap=tokmap_sb[:, ci:ci + 1], axis=0),
                    )
                    for dk in range(DK):
                        ps_tr = trp.tile([128, 128], f16)
                        nc.tensor.transpose(ps_tr[:], dchunk[:, dk * 128:(dk + 1) * 128],
                                            ident16[:])
                        nc.vector.tensor_copy(
                            dispT[:, dk * C_PAD + ci * 128: dk * C_PAD + (ci + 1) * 128],
                            ps_tr[:])

            # ================= PHASE F: FFN =================
            b1c = cpool.tile([128, HQ], f32)
            nc.sync.dma_start(out=b1c[:], in_=b1v.rearrange("(hq p) -> p hq", p=128))
            b2row = cpool.tile([1, D], f32)
            nc.sync.dma_start(out=b2row[:], in_=b2v.rearrange("(u d) -> u d", u=1))
            zero16 = cpool.tile([1, D], f16)
            nc.vector.memset(zero16[:], 0.0)
            nc.sync.dma_start(out=y_dram[C_PAD:C_PAD + 1, :], in_=zero16[:])

            c_sups = [(0, 512), (512, 512), (1024, 256)]
            with (
                tc.tile_pool(name="hT", bufs=1) as hpool,
                tc.tile_pool(name="w1s", bufs=24) as w1p,
                tc.tile_pool(name="w2s", bufs=6) as w2p,
                tc.tile_pool(name="psh", bufs=1, space="PSUM") as pshp,
                tc.tile_pool(name="psy", bufs=1, space="PSUM") as psyp,
                tc.tile_pool(name="ysb", bufs=4) as yp,
            ):
                hT = hpool.tile([128, HQ * 512], f16)
                for cs, cw in c_sups:
                    # FFN1: h_T[h', c] = relu(sum_d w1[d, h'] * dispT[d, c] + b1)
                    for hqb in range(HQ // 4):  # blocks of 4 h'-chunks
                        ps_hs = [pshp.tile([128, 512], f32, tag=f"ps_h{j}",
                                           name=f"ps_h{j}")
                                 for j in range(4)]
                        for dk in range(DK):
                            w1t = w1p.tile([128, 512], f16, tag="w1t")
                            nc.sync.dma_start(out=w1t[:], in_=w1c[dk, hqb])
                            for j in range(4):
                                nc.tensor.matmul(
                                    ps_hs[j][:, 0:cw],
                                    w1t[:, j * 128:(j + 1) * 128],
                                    dispT[:, dk * C_PAD + cs: dk * C_PAD + cs + cw],
                                    start=(dk == 0), stop=(dk == DK - 1))
                        for j in range(4):
                            hq = hqb * 4 + j
                            nc.scalar.activation(hT[:, hq * 512: hq * 512 + cw],
                                                 ps_hs[j][:, 0:cw], ACT.Relu,
                                                 bias=b1c[:, hq:hq + 1])
                    # FFN2: y[c, d] = sum_h' h_T[h', c] * w2[h', d] + b2
                    ncc = cw // 128
                    for dg in range(4):
                        ps_ys = [psyp.tile([128, 512], f32, tag=f"ps_y{cc}",
                                           name=f"ps_y{cc}")
                                 for cc in range(ncc)]
                        for hk in range(HQ):
                            w2t = w2p.tile([128, 512], f16, tag="w2t")
                            nc.scalar.dma_start(out=w2t[:], in_=w2c[hk, dg])
                            for cc in range(ncc):
                                nc.tensor.matmul(
                                    ps_ys[cc][:],
                                    hT[:, hk * 512 + cc * 128: hk * 512 + (cc + 1) * 128],
                                    w2t[:],
                                    start=(hk == 0), stop=False)
                        for cc in range(ncc):
                            nc.tensor.matmul(ps_ys[cc][:], ones_1x128[:],
                                             b2row[:, dg * 512:(dg + 1) * 512],
                                             start=False, stop=True)
                            y_sb = yp.tile([128, 512], f16, tag="y_sb")
                            nc.vector.tensor_copy(y_sb[:], ps_ys[cc][:])
                            nc.sync.dma_start(
                                out=y_dram[cs + cc * 128: cs + (cc + 1) * 128,
                                           dg * 512:(dg + 1) * 512],
                                in_=y_sb[:])

            # ========= PHASE D: decode + chunked AllReduce overlap =========
            AR_CHUNK = 4  # token-blocks per AllReduce slab (4*128 rows, f16)
            with (
                tc.tile_pool(name="yt", bufs=3) as ytp,
                tc.tile_pool(name="op", bufs=3) as opp,
                tc.tile_pool(name="arc", bufs=3) as arp,
            ):
                for g in range(NB):
                    yt = ytp.tile([128, D], f16, tag="yt")
                    nc.gpsimd.indirect_dma_start(
                        out=yt[:], out_offset=None,
                        in_=y_dram[:], in_offset=bass.IndirectOffsetOnAxis(
                            ap=slot_i[:, g:g + 1], axis=0),
                    )
                    op_t = opp.tile([128, D], f16, tag="op_t")
                    nc.vector.tensor_scalar(op_t[:], yt[:], coeff[:, g:g + 1],
                                            None, AO.mult)
                    nc.sync.dma_start(out=partial_d[g * 128:(g + 1) * 128, :],
                                      in_=op_t[:])
                for ch in range(NB // AR_CHUNK):
                    r0, r1 = ch * AR_CHUNK * 128, (ch + 1) * AR_CHUNK * 128
                    nc.gpsimd.collective_compute(
                        "AllReduce", AO.add,
                        replica_groups=[list(range(N_CORES))],
                        ins=[partial_d[r0:r1, :]], outs=[ar_out[r0:r1, :]],
                    )
                    # f16 -> f32 cast through SBUF, then to the output
                    for gg in range(ch * AR_CHUNK, (ch + 1) * AR_CHUNK):
                        ar_sb = arp.tile([128, D], f16, tag="ar_sb")
                        nc.sync.dma_start(
                            out=ar_sb[:], in_=ar_out[gg * 128:(gg + 1) * 128, :])
                        ar_f32 = arp.tile([128, D], f32, tag="ar_f32")
                        nc.vector.tensor_copy(ar_f32[:], ar_sb[:])
                        nc.sync.dma_start(
                            out=out[gg * 128:(gg + 1) * 128, :], in_=ar_f32[:])

    nc.compile()
    _BUILD_CACHE["nc"] = nc
    return nc


def _prep_inputs(x, wg, w1, b1, w2, b2):
    x = np.asarray(x, np.float32)
    wg = np.asarray(wg, np.float32)
    w1 = np.asarray(w1, np.float32)
    b1 = np.asarray(b1, np.float32)
    w2 = np.asarray(w2, np.float32)
    b2 = np.asarray(b2, np.float32)

    xT = np.ascontiguousarray(x.T)  # [D, T]
    xt_hi = xT.astype(ml_dtypes.bfloat16)
    xt_lo = (xT - xt_hi.astype(np.float32)).astype(ml_dtypes.bfloat16)
    # chunk layout [dk, g, 128, 512]
    xt_hi = np.ascontiguousarray(
        xt_hi.reshape(16, 128, 8, 512).transpose(0, 2, 1, 3))
    xt_lo = np.ascontiguousarray(
        xt_lo.reshape(16, 128, 8, 512).transpose(0, 2, 1, 3))
    wg_hi = wg.astype(ml_dtypes.bfloat16)
    wg_lo = (wg - wg_hi.astype(np.float32)).astype(ml_dtypes.bfloat16)
    x16 = x.astype(np.float16)

    shared = {
        "xt_hi": xt_hi, "xt_lo": xt_lo,
        "wg_hi": wg_hi, "wg_lo": wg_lo, "x16": x16,
    }
    in_maps = []
    for e in range(N_CORES):
        w1e = w1[e].astype(np.float16)  # [D, H]
        w1ce = np.ascontiguousarray(
            w1e.reshape(16, 128, 16, 512).transpose(0, 2, 1, 3))
        w2e = w2[e].astype(np.float16)  # [H, D]
        w2ce = np.ascontiguousarray(
            w2e.reshape(64, 128, 4, 512).transpose(0, 2, 1, 3))
        m = dict(shared)
        m["w1c"] = w1ce
        m["w2c"] = w2ce
        m["b1v"] = np.ascontiguousarray(b1[e, 0])
        m["b2v"] = np.ascontiguousarray(b2[e, 0])
        m["eidt"] = np.full((128, 1), float(e), np.float32)
        in_maps.append(m)
    return in_maps


def kernel(x, wg, w1, b1, w2, b2, k=2, _want_results=False, _trace=False, **_ignored):
    assert int(k) == K
    nc = build_bass()
    in_maps = _prep_inputs(x, wg, w1, b1, w2, b2)
    res = run_bass_kernel_spmd(nc, in_maps, core_ids=list(range(N_CORES)),
                               trace=_trace)
    out = np.asarray(res.results[0]["out"])
    laux = np.float32(np.asarray(res.results[0]["laux"])[0])
    if _want_results:
        return (out, laux), res
    return out, laux
